# revision 56
# baseline (speedup 1.0000x reference)
"""DiffPool-like GNN (two GCN convs + softmax clustering + weighted pooling)
as a Bass/Tile SPMD kernel on 8 Trainium2 NeuronCores.

Strategy (matches the sharding hint):
  * nodes partitioned into 8 contiguous shards; each core owns the edges whose
    dst falls in its shard (host buckets edges by 128-node dst window);
  * W1/W2 replicated; per-core f32 feature tables g = (D^-1/2 x) @ W are
    built on-device and stored in HBM in a per-core permuted row order (own
    shard first) so one SPMD program serves all cores;
  * per-edge messages fetched with the MoE dma_gather primitive (256B rows,
    int16 indices -> the table is split into 4 parts of Npad/4 rows, each with
    a trailing zero row for padding slots); edges are laid out PART-MAJOR so
    each gather instruction covers many dst windows of one part (large
    gathers amortize the ~1us fixed SWDGE descriptor-generation cost);
  * segment-sum on the tensor engine: per 128-slot chunk, a one-hot matrix
    M[p, r] = (dstloc[p] == r) is built on the vector engine and
    M.T @ msgs accumulates in PSUM per (part, window); partial sums across
    parts accumulate in an SBUF f32 tile agg[:, w*F:(w+1)*F];
  * conv output x = dinv*(agg + g_self) + b, emitted in the last part's pass;
  * two launches: (A) conv1 -> x rows (and xs = dinv*x for the next table);
    host regroups; (B) conv2 -> softmax S -> weighted pooling via the same
    masked-matmul trick, emitting per-core partial pooled sums which the host
    adds (the "all-reduce").

The walrus build in this container encodes at most ONE sync wait per
instruction; _split_waits() rewrites the scheduled BIR, moving excess waits
onto injected single-wait NoOps.
"""

import os
import sys
import numpy as np

sys.path.insert(0, "/opt/trn_rl_repo")

import ml_dtypes  # noqa: E402
import concourse.bacc as bacc  # noqa: E402
import concourse.mybir as mybir  # noqa: E402
import concourse.tile as tile  # noqa: E402
from concourse.bass_utils import run_bass_kernel_spmd  # noqa: E402
from concourse.tile_rust import add_dep_helper  # noqa: E402

P = 128
BF16 = mybir.dt.bfloat16
F32 = mybir.dt.float32
I16 = mybir.dt.int16
I32 = mybir.dt.int32
NP_BF16 = ml_dtypes.bfloat16

AluOp = mybir.AluOpType
ActFn = mybir.ActivationFunctionType

_DT_MAP = {
    np.dtype(np.float32): F32,
    np.dtype(np.int16): I16,
    np.dtype(NP_BF16): BF16,
}

PARTS = 4
MAXCH = 8  # max 128-row chunks per dma_gather instruction (1024 rows, HW max)
NQ = 4     # SWDGE queues; round-robin gathers so ring drains overlap


class ConstBundle:
    """Packs [128, n] arrays of mixed dtypes into one [128, W] int32 array."""

    def __init__(self):
        self.fields = {}
        self.nbytes = 0

    def add(self, name, dtype, n):
        dt = np.dtype(dtype)
        b = dt.itemsize * n
        b4 = (b + 3) & ~3
        self.fields[name] = (self.nbytes, dt, n)
        self.nbytes += b4

    def pack(self, arrays):
        w = self.nbytes // 4
        out = np.zeros((P, w), np.int32)
        ob = out.view(np.uint8)
        for name, (off, dt, n) in self.fields.items():
            a = np.ascontiguousarray(arrays[name])
            assert a.dtype == dt and a.shape == (P, n), (name, a.dtype, a.shape)
            ob[:, off:off + dt.itemsize * n] = a.view(np.uint8)
        return out

    def view(self, cb_sb, name):
        off, dt, n = self.fields[name]
        b4 = (dt.itemsize * n + 3) & ~3
        v = cb_sb[:, off // 4:(off + b4) // 4].bitcast(_DT_MAP[dt])
        return v[:, :n]


def _split_waits(nc, budget=1):
    """Move excess sync waits onto injected single-wait same-engine NoOps.
    The walrus in this container encodes at most one wait per instruction."""
    for fn in nc.m.functions:
        for blk in fn.blocks:
            out = []
            for ins in blk.instructions:
                si = ins.sync_info
                if (si is not None and si.on_wait
                        and len(si.on_wait) > budget
                        and ins.opcode not in ("EventSemaphore",)):
                    waits = list(si.on_wait)
                    excess, keep = waits[:-budget], waits[-budget:]
                    for i, wv in enumerate(excess):
                        nop = mybir.InstNoOp(
                            name=f"{ins.name}-sw{i}", engine=ins.engine,
                            bass_nofuse=True,
                            sync_info=mybir.SyncInfo(on_wait=[wv], on_update=[]))
                        out.append(nop)
                    si.on_wait = keep
                out.append(ins)
            blk.instructions[:] = out


def _wrap16(flat):
    """dma_gather index layout: [128, n/16] int16; index j sits at
    [16*g + j%16, j//16], replicated for all 8 groups g."""
    n = flat.shape[0]
    assert n % 16 == 0
    base = flat.reshape(n // 16, 16).T.astype(np.int16)
    return np.tile(base, (8, 1))


def _iota_full():
    return np.tile(np.arange(P, dtype=NP_BF16)[None, :], (P, 1))


# =========================================================================
# host-side preprocessing
# =========================================================================

class Meta:
    pass


def preprocess(x_in, edge_index, batch, W1, b1, W2, b2, n_cores=8):
    pr = Meta()
    N, IN = x_in.shape
    D = W1.shape[1]
    K = W2.shape[1]

    src = np.ascontiguousarray(edge_index[0]).astype(np.int64)
    dst = np.ascontiguousarray(edge_index[1]).astype(np.int64)
    batch = np.asarray(batch).astype(np.int64)

    WPC = int(np.ceil(N / n_cores / P))
    NS = WPC * P
    Npad = NS * n_cores
    NT = Npad // P
    assert Npad % PARTS == 0
    PS = Npad // PARTS
    PSZ = PS + P
    TPG = PS // P
    TB = None
    for cand in (7, 8, 4, 2, 1):
        if TPG % cand == 0:
            TB = cand
            break
    pr.TB = TB

    deg = np.bincount(dst, minlength=N).astype(np.float64)
    dinv_pad = np.ones(Npad, np.float32)
    dinv_pad[:N] = (1.0 / np.sqrt(deg + 1.0)).astype(np.float32)

    pr.__dict__.update(dict(
        N=N, B=128, IN=IN, D=D, K=K, n_cores=n_cores, WPC=WPC, NS=NS,
        Npad=Npad, NT=NT, PS=PS, PSZ=PSZ, dinv_pad=dinv_pad,
        W1=W1.astype(np.float32), b1=b1.astype(np.float32),
        W2=W2.astype(np.float32), b2=b2.astype(np.float32),
    ))

    xs_pad = np.zeros((Npad, IN), np.float32)
    xs_pad[:N] = x_in * dinv_pad[:N, None]

    # ---- per-core permutation + edge slotting by (table part, dst window)
    pr.perm, pr.xT_A, pr.dinvT = [], [], []
    core_srcloc, core_key, core_dstloc, core_cnt = [], [], [], []
    maxcnt = 0
    for c in range(n_cores):
        shard = np.arange(c * NS, (c + 1) * NS)
        others = np.concatenate(
            [np.arange(0, c * NS), np.arange((c + 1) * NS, Npad)])
        perm = np.concatenate([shard, others])
        rowpos = np.empty(Npad, np.int64)
        rowpos[perm] = np.arange(Npad)
        pr.perm.append(perm)
        pr.xT_A.append(np.ascontiguousarray(xs_pad[perm].T).astype(NP_BF16))
        pr.dinvT.append(np.ascontiguousarray(
            dinv_pad[shard].reshape(WPC, P).T))

        sel = (dst >= c * NS) & (dst < (c + 1) * NS)
        es, ed = src[sel], dst[sel]
        erow = rowpos[es]
        part = erow // PS
        wloc = (ed - c * NS) // P
        key = part * WPC + wloc          # PART-MAJOR stream order
        srcloc = erow % PS
        order = np.lexsort((srcloc, key))  # in-bucket ascending srcloc
        cnt = np.bincount(key, minlength=PARTS * WPC)
        core_srcloc.append(srcloc[order])
        core_dstloc.append(
            (ed[order] - c * NS - wloc[order] * P).astype(np.float32))
        core_key.append(key[order])
        core_cnt.append(cnt)

    # ---- shared straddle-packed stream schedule: per-(part, window) slot
    # capacity = max count across cores; window slot ranges are contiguous
    # and NOT chunk-aligned -- boundary chunks serve two windows via two
    # masked one-hot occurrences.
    caps = np.stack(core_cnt).max(axis=0)          # [PARTS*WPC]
    assert caps.min() > 0
    stream_starts = np.empty(PARTS * WPC, np.int64)
    part_base, CHs = [], []
    off = 0
    for q in range(PARTS):
        seg = caps[q * WPC:(q + 1) * WPC]
        stream_starts[q * WPC:(q + 1) * WPC] = (
            off + np.concatenate([[0], np.cumsum(seg)[:-1]]))
        L128 = -(-int(seg.sum()) // P) * P
        part_base.append(off)
        CHs.append(L128 // P)
        off += L128
    TOT = off
    pr.ech = CHs
    pr.TOTCH = TOT // P

    win_id = np.full(TOT, -1, np.int64)
    for q in range(PARTS):
        seg = caps[q * WPC:(q + 1) * WPC]
        L = int(seg.sum())
        win_id[part_base[q]:part_base[q] + L] = np.repeat(
            np.arange(q * WPC, (q + 1) * WPC), seg)

    pr.ewin = []
    occ_g, occ_w = [], []
    col = 0
    for q in range(PARTS):
        lst = []
        for w in range(WPC):
            s = int(stream_starts[q * WPC + w])
            e = s + int(caps[q * WPC + w])
            g0, g1 = s // P, (e - 1) // P
            lst.append((col, g0, g1))
            for g in range(g0, g1 + 1):
                occ_g.append(g)
                occ_w.append(q * WPC + w)
            col += g1 - g0 + 1
        pr.ewin.append(lst)
    NOCC = col
    pr.NOCC = NOCC
    pr.NOCC_MAX = max(g1 - g0 + 1 for lst in pr.ewin for (_, g0, g1) in lst)
    occ_g = np.asarray(occ_g)
    occ_w = np.asarray(occ_w)
    win_blk = win_id.reshape(-1, P)[occ_g]         # [NOCC, 128]

    grpstart = np.zeros(PARTS * WPC + 1, np.int64)
    pr.srcg16, pr.dstlocT = [], []
    for c in range(n_cores):
        key, cnt = core_key[c], core_cnt[c]
        np.cumsum(cnt, out=grpstart[1:])
        pos = np.arange(len(key)) - grpstart[key]
        slot_idx = stream_starts[key] + pos
        slots = np.full(TOT, PS, np.int64)
        dloc = np.full(TOT, -1.0, np.float32)
        slots[slot_idx] = core_srcloc[c]
        dloc[slot_idx] = core_dstloc[c]
        pr.srcg16.append(_wrap16(slots))
        cols = np.where(win_blk == occ_w[:, None],
                        dloc.reshape(-1, P)[occ_g], -1.0)
        pr.dstlocT.append(
            np.ascontiguousarray(cols.T).astype(NP_BF16))

    # ---- pooling prep
    GW, NPW = 4, 8
    pr.GW, pr.NPW = GW, NPW
    pr.gbase = []
    core_win_nodes = []
    tpw = 1
    for c in range(n_cores):
        lo, hi = c * NS, min((c + 1) * NS, N)
        if lo >= N:
            pr.gbase.append(0)
            core_win_nodes.append([np.empty(0, np.int64)] * NPW)
            continue
        gb = int(batch[lo])
        assert int(batch[hi - 1]) - gb + 1 <= NPW * GW
        pr.gbase.append(gb)
        nodes = np.arange(lo, hi)
        gl = batch[lo:hi] - gb
        wins = []
        for w2 in range(NPW):
            sel = nodes[(gl >= w2 * GW) & (gl < (w2 + 1) * GW)]
            wins.append(sel)
            tpw = max(tpw, int(np.ceil(len(sel) / P)))
        core_win_nodes.append(wins)
    pr.TPW, pr.PT = tpw, NPW * tpw

    pr.poolidx16, pr.batchlocT, pr.poolnode = [], [], []
    for c in range(n_cores):
        pidx = np.zeros(pr.PT * P, np.int64)
        bloc = np.full((P, pr.PT), -1.0, np.float32)
        pnode = np.full(pr.PT * P, -1, np.int64)
        for w2 in range(NPW):
            sel = core_win_nodes[c][w2]
            for t in range(tpw):
                tt = w2 * tpw + t
                seg = sel[t * P:(t + 1) * P]
                n = len(seg)
                if n:
                    pidx[tt * P:tt * P + n] = seg - c * NS
                    bloc[:n, tt] = (batch[seg] - pr.gbase[c] - w2 * GW)
                    pnode[tt * P:tt * P + n] = seg
        pr.poolidx16.append(_wrap16(pidx))
        pr.batchlocT.append(bloc.astype(NP_BF16))
        pr.poolnode.append(pnode)

    # ---- const bundles (layout shared across cores)
    pr.cbA = ConstBundle()
    pr.cbA.add("dinvT", np.float32, WPC)
    pr.cbA.add("srcg", np.int16, pr.TOTCH * 8)
    pr.cbA.add("bt", np.float32, D)
    pr.cbA.add("w1", NP_BF16, D)
    pr.cbA.add("iota", NP_BF16, P)
    pr.cbA.add("dstloc", NP_BF16, pr.NOCC)

    pr.cbB = ConstBundle()
    pr.cbB.add("dinvT", np.float32, WPC)
    pr.cbB.add("srcg", np.int16, pr.TOTCH * 8)
    pr.cbB.add("bt", np.float32, K)
    pr.cbB.add("poolidx", np.int16, pr.PT * 8)
    pr.cbB.add("w2", NP_BF16, D)
    pr.cbB.add("iota", NP_BF16, P)
    pr.cbB.add("dstloc", NP_BF16, pr.NOCC)
    pr.cbB.add("bloc", NP_BF16, pr.PT)
    return pr


# =========================================================================
# Bass program builders
# =========================================================================

def _build_table(nc, pools, pr, xT_d, w_sb, gtab, g_shard, FIN, FOUT,
                 FSH=None, tb=None):
    """h-table build: per part, TB-tile groups; f32 rows -> gtab + zero rows.
    w_sb is pre-padded to FOUT columns; g_shard keeps only FSH (<= FOUT) of
    them. Returns per-part write lists for gather dependency tracking."""
    FSH = FOUT if FSH is None else FSH
    TB, WPC, PS = tb or pr.TB, pr.WPC, pr.PS
    TPG = PS // P
    writes = [[] for _ in range(PARTS)]
    xtp, pp8, g8p = pools["xt"], pools["ps8"], pools["g8"]
    zf = pools["const"].tile([P, FOUT], F32, name="zf_sb", tag="zf_sb")
    nc.vector.memset(zf[:], 0)
    for q in range(PARTS):
        for gi in range(TPG // TB):
            t0 = q * TPG + gi * TB
            xt = xtp.tile([FIN, TB * P], BF16)
            nc.sync.dma_start(out=xt[:], in_=xT_d[:, t0 * P:(t0 + TB) * P])
            ps = pp8.tile([P, TB * FOUT], F32)
            for j in range(TB):
                nc.tensor.matmul(ps[:, j * FOUT:(j + 1) * FOUT],
                                 lhsT=xt[:, j * P:(j + 1) * P],
                                 rhs=w_sb[:], start=True, stop=True)
            g8 = g8p.tile([P, TB * FOUT], F32)
            nc.scalar.copy(out=g8[:], in_=ps[:])
            lo = t0
            if lo < WPC:
                nj = min(WPC - lo, TB)
                if FSH == FOUT:
                    nc.vector.tensor_copy(
                        out=g_shard[:, lo * FOUT:(lo + nj) * FOUT],
                        in_=ps[:, :nj * FOUT])
                else:
                    nc.vector.tensor_copy(
                        out=g_shard[:, lo * FSH:(lo + nj) * FSH]
                            .rearrange("p (t d) -> p t d", d=FSH),
                        in_=ps[:, :nj * FOUT]
                            .rearrange("p (t d) -> p t d", d=FOUT)[:, :, :FSH])
            w = nc.sync.dma_start(
                out=gtab[q * pr.PSZ + gi * TB * P:
                         q * pr.PSZ + (gi + 1) * TB * P, :]
                    .rearrange("(t p) d -> p t d", p=P),
                in_=g8[:].rearrange("p (t d) -> p t d", d=FOUT))
            writes[q].append(w)
        wz = nc.sync.dma_start(out=gtab[q * pr.PSZ + PS:(q + 1) * pr.PSZ, :],
                               in_=zf[:])
        writes[q].append(wz)
    return writes


def _edge_phase(nc, pools, pr, gtab, srcg_sb, dstloc_sb, iota_sb,
                writes_by_part, FPAD, FUSE, agg_sb, finish, probe=None):
    """Part-major straddle-packed gather stream in full 8-chunk (1024-row,
    HW max) gathers. Window slot ranges are contiguous but not chunk-aligned;
    a boundary chunk feeds two windows via two masked one-hot occurrences.
    Per (part, window): one matmul per occurrence accumulates in PSUM, then
    folds into the SBUF f32 accumulator. finish(w) fires during the last
    part's pass."""
    WPC = pr.WPC
    GC = MAXCH
    msp, mqp, mtp, pp = pools["msgs"], pools["msq"], pools["mt"], pools["ps"]
    mt_const = [None]
    gbase = 0
    gcount = 0
    for q in range(PARTS):
        CH = pr.ech[q]
        wlist = pr.ewin[q]
        msqs = {}
        w_next = 0
        first = True
        for t0 in range(0, CH, GC):
            nk = min(GC, CH - t0)
            g0 = gbase + t0
            msgs = msp.tile([P, GC * FPAD], F32)
            g = nc.gpsimd.dma_gather(
                msgs[:, :nk * FPAD].rearrange("p (c e) -> p c e", e=FPAD),
                gtab[q * pr.PSZ:(q + 1) * pr.PSZ, :],
                srcg_sb[:, g0 * 8:(g0 + nk) * 8],
                nk * P, nk * P, FPAD, single_packet=False,
                queue_num=gcount % NQ)
            gcount += 1
            if first:
                first = False
                for tw in writes_by_part[q]:
                    add_dep_helper(g.ins, tw.ins, sync=True,
                                   reason="gather after table part")
            msq = mqp.tile([P, GC * FPAD], BF16)
            nc.scalar.copy(out=msq[:, :nk * FPAD], in_=msgs[:, :nk * FPAD])
            for i in range(nk):
                msqs[g0 + i] = (msq, i)
            if probe == "gather":
                continue
            avail = g0 + nk
            while w_next < WPC and wlist[w_next][2] < avail:
                col0, gg0, gg1 = wlist[w_next]
                w = w_next
                w_next += 1
                nocc = gg1 - gg0 + 1
                if probe == "noonehot":
                    if mt_const[0] is None:
                        mt_const[0] = pools["const"].tile(
                            [P, pr.NOCC_MAX * P], BF16, name="mt_c",
                            tag="mt_c")
                        nc.vector.memset(mt_const[0][:], 0)
                    mt = mt_const[0]
                else:
                    mt = mtp.tile([P, pr.NOCC_MAX * P], BF16)
                    nc.vector.tensor_tensor(
                        out=mt[:, :nocc * P].rearrange("p (k r) -> p k r", r=P),
                        in0=dstloc_sb[:, col0:col0 + nocc]
                            .unsqueeze(2).to_broadcast([P, nocc, P]),
                        in1=iota_sb[:].unsqueeze(1).to_broadcast([P, nocc, P]),
                        op=AluOp.is_equal)
                ps = pp.tile([P, FUSE], F32)
                for ci in range(nocc):
                    mq, sl = msqs[gg0 + ci]
                    nc.tensor.matmul(
                        ps[:],
                        lhsT=mt[:, ci * P:(ci + 1) * P],
                        rhs=mq[:, sl * FPAD:sl * FPAD + FUSE],
                        start=(ci == 0), stop=(ci == nocc - 1))
                aw = agg_sb[:, w * FUSE:(w + 1) * FUSE]
                if q == 0:
                    nc.vector.tensor_copy(out=aw, in_=ps[:])
                else:
                    nc.vector.tensor_tensor(out=aw, in0=aw, in1=ps[:],
                                            op=AluOp.add)
                if q == PARTS - 1 and probe != "nofinish":
                    finish(w, aw)
        gbase += CH


def _mk_pools(tc, es, extra=(), depth=16):
    pools = {}
    names = [("const", 1, None), ("xt", 3, None), ("g8", 4, None),
             ("msgs", depth, None), ("msq", depth, None), ("mt", 4, None),
             ("xw", 2, None), ("st", 2, None),
             ("ps8", 2, "PSUM"), ("ps", 4, "PSUM")]
    names += list(extra)
    for nm, bufs, space in names:
        kw = dict(name=nm, bufs=bufs)
        if space:
            kw["space"] = space
        pools[nm] = es.enter_context(tc.tile_pool(**kw))
    return pools


def build_A(pr, split=True, repeat=1, probe=None):
    from contextlib import ExitStack
    IN, D, WPC, Npad, NS = pr.IN, pr.D, pr.WPC, pr.Npad, pr.NS
    CBW = pr.cbA.nbytes // 4
    SW = 7
    assert WPC % SW == 0

    nc = bacc.Bacc("TRN2", num_swdge_queues=NQ)
    xT_d = nc.declare_dram_parameter("xT", [IN, Npad], BF16, isOutput=False)
    cb_d = nc.declare_dram_parameter("cb", [P, CBW], I32, isOutput=False)
    xout_d = nc.declare_dram_parameter("xout", [NS, D], BF16, isOutput=True)
    xsout_d = nc.declare_dram_parameter("xsout", [NS, D], BF16, isOutput=True)
    gtab = nc.dram_tensor("gtab", [PARTS * pr.PSZ, D], F32)

    with tile.TileContext(nc) as tc, ExitStack() as es:
        pools = _mk_pools(tc, es, depth=12)
        cp = pools["const"]
        cb_sb = cp.tile([P, CBW], I32, name="cb_sb", tag="cb_sb")
        nc.sync.dma_start(out=cb_sb[:], in_=cb_d[:])
        nc.vector.tensor_copy(out=cb_sb[:], in_=cb_sb[:])
        V = lambda name: pr.cbA.view(cb_sb, name)
        g_shard = cp.tile([P, WPC * D], F32)
        agg_sb = cp.tile([P, WPC * D], F32, name="agg_sb", tag="agg_sb")

        dinvT_sb, bt_sb, iota_sb = V("dinvT"), V("bt"), V("iota")
        xwp, stp = pools["xw"], pools["st"]
        stage = {}

        def finish(w, aw):
            # batched: act once per SW completed windows on contiguous agg
            if w % SW != SW - 1:
                return
            w0 = w - SW + 1
            agg7 = agg_sb[:, w0 * D:(w0 + SW) * D]
            dib = (dinvT_sb[:, w0:w0 + SW].unsqueeze(2)
                   .to_broadcast([P, SW, D]))
            t17 = xwp.tile([P, SW * D], F32, tag="t17")
            nc.vector.tensor_tensor(out=t17[:], in0=agg7,
                                    in1=g_shard[:, w0 * D:(w0 + SW) * D],
                                    op=AluOp.add)
            xf7 = xwp.tile([P, SW * D], F32, tag="xf7")
            nc.vector.tensor_tensor(
                out=xf7[:].rearrange("p (w d) -> p w d", d=D),
                in0=t17[:].rearrange("p (w d) -> p w d", d=D),
                in1=dib, op=AluOp.mult)
            xq7 = stp.tile([P, SW * D], BF16, name="xq_st", tag="xq_st")
            nc.vector.tensor_tensor(
                out=xq7[:].rearrange("p (w d) -> p w d", d=D),
                in0=xf7[:].rearrange("p (w d) -> p w d", d=D),
                in1=bt_sb[:].unsqueeze(1).to_broadcast([P, SW, D]),
                op=AluOp.add)
            xsq7 = stp.tile([P, SW * D], BF16, name="xsq_st", tag="xsq_st")
            nc.vector.tensor_tensor(
                out=xsq7[:].rearrange("p (w d) -> p w d", d=D),
                in0=xq7[:].rearrange("p (w d) -> p w d", d=D),
                in1=dib, op=AluOp.mult)
            nc.sync.dma_start(
                out=xout_d[w0 * P:(w0 + SW) * P, :]
                    .rearrange("(t p) d -> p t d", p=P),
                in_=xq7[:].rearrange("p (t d) -> p t d", d=D))
            nc.sync.dma_start(
                out=xsout_d[w0 * P:(w0 + SW) * P, :]
                    .rearrange("(t p) d -> p t d", p=P),
                in_=xsq7[:].rearrange("p (t d) -> p t d", d=D))

        for _ in range(repeat):
            writes = _build_table(nc, pools, pr, xT_d, V("w1"), gtab,
                                  g_shard, IN, D, tb=14)
            _edge_phase(nc, pools, pr, gtab, V("srcg"), V("dstloc"), iota_sb,
                        writes, D, D, agg_sb, finish, probe=probe)
    nc.compile()
    if split:
        _split_waits(nc)
    return nc


def build_B(pr, split=True, repeat=1):
    from contextlib import ExitStack
    D, K, WPC, Npad, NS = pr.D, pr.K, pr.WPC, pr.Npad, pr.NS
    GW, NPW, TPW, PT = pr.GW, pr.NPW, pr.TPW, pr.PT
    CBW = pr.cbB.nbytes // 4

    nc = bacc.Bacc("TRN2", num_swdge_queues=NQ)
    xT2_d = nc.declare_dram_parameter("xT2", [D, Npad], BF16, isOutput=False)
    cb_d = nc.declare_dram_parameter("cb", [P, CBW], I32, isOutput=False)
    xpool_d = nc.declare_dram_parameter("xpool", [PT * P, D], BF16, isOutput=False)
    pool_d = nc.declare_dram_parameter("pool", [P, NPW * D], F32, isOutput=True)
    gtab = nc.dram_tensor("g2tab", [PARTS * pr.PSZ, D], F32)
    s_hbm = nc.dram_tensor("s_hbm", [NS + P, D], F32)

    with tile.TileContext(nc) as tc, ExitStack() as es:
        pools = _mk_pools(tc, es, extra=[
            ("sw", 4, None), ("xp", 2, None), ("spl", 2, None),
            ("plp", 2, "PSUM")], depth=10)
        cp = pools["const"]
        cb_sb = cp.tile([P, CBW], I32, name="cb_sb", tag="cb_sb")
        nc.sync.dma_start(out=cb_sb[:], in_=cb_d[:])
        nc.vector.tensor_copy(out=cb_sb[:], in_=cb_sb[:])
        V = lambda name: pr.cbB.view(cb_sb, name)
        g_shard = cp.tile([P, WPC * K], F32)
        agg_sb = cp.tile([P, WPC * K], F32, name="agg_sb", tag="agg_sb")

        dinvT_sb, bt_sb, iota_sb = V("dinvT"), V("bt"), V("iota")
        poolidx_sb, bloc_sb = V("poolidx"), V("bloc")
        swp = pools["sw"]

        s_sb = cp.tile([P, WPC * D], F32)
        nc.vector.memset(s_sb[:], 0)

        SWB = 7
        assert WPC % SWB == 0

        def finish(w, aw):
            # batched softmax once per SWB completed windows;
            # bt_sb holds exp(b2): softmax(z+b) = e^z * e^b / sum(...)
            if w % SWB != SWB - 1:
                return
            w0 = w - SWB + 1
            agg7 = agg_sb[:, w0 * K:(w0 + SWB) * K]
            t17 = swp.tile([P, SWB * K], F32, tag="t17")
            nc.vector.tensor_tensor(out=t17[:], in0=agg7,
                                    in1=g_shard[:, w0 * K:(w0 + SWB) * K],
                                    op=AluOp.add)
            z7 = swp.tile([P, SWB * K], F32, tag="z7")
            nc.vector.tensor_tensor(
                out=z7[:].rearrange("p (w k) -> p w k", k=K),
                in0=t17[:].rearrange("p (w k) -> p w k", k=K),
                in1=(dinvT_sb[:, w0:w0 + SWB].unsqueeze(2)
                     .to_broadcast([P, SWB, K])),
                op=AluOp.mult)
            ex7 = swp.tile([P, SWB * K], F32, tag="ex7")
            nc.scalar.activation(out=ex7[:], in_=z7[:], func=ActFn.Exp)
            eb7 = swp.tile([P, SWB * K], F32, tag="eb7")
            nc.vector.tensor_tensor(
                out=eb7[:].rearrange("p (w k) -> p w k", k=K),
                in0=ex7[:].rearrange("p (w k) -> p w k", k=K),
                in1=bt_sb[:].unsqueeze(1).to_broadcast([P, SWB, K]),
                op=AluOp.mult)
            sm7 = swp.tile([P, SWB], F32, tag="sm7")
            nc.vector.tensor_reduce(
                out=sm7[:].unsqueeze(2),
                in_=eb7[:].rearrange("p (w k) -> p w k", k=K),
                axis=mybir.AxisListType.X, op=AluOp.add)
            rc7 = swp.tile([P, SWB], F32, tag="rc7")
            nc.vector.reciprocal(out=rc7[:], in_=sm7[:])
            nc.vector.tensor_tensor(
                out=s_sb[:, w0 * D:(w0 + SWB) * D]
                    .rearrange("p (w d) -> p w d", d=D)[:, :, :K],
                in0=eb7[:].rearrange("p (w k) -> p w k", k=K),
                in1=rc7[:].unsqueeze(2).to_broadcast([P, SWB, K]),
                op=AluOp.mult)

        for _ in range(repeat):
            writes = _build_table(nc, pools, pr, xT2_d, V("w2")[:D, :],
                                  gtab, g_shard, D, D, FSH=K)
            _edge_phase(nc, pools, pr, gtab, V("srcg"), V("dstloc"), iota_sb,
                        writes, D, K, agg_sb, finish)

        # ---- pooling
        zs = cp.tile([P, D], F32, name="zs_sb", tag="zs_sb")
        nc.vector.memset(zs[:], 0)
        s_write2 = nc.sync.dma_start(out=s_hbm[NS:NS + P, :], in_=zs[:])
        s_write = nc.sync.dma_start(
            out=s_hbm[:NS, :].rearrange("(w p) k -> p w k", p=P),
            in_=s_sb[:].rearrange("p (w k) -> p w k", k=D))
        pool_sb = cp.tile([P, NPW * D], F32)
        splp, xpp, mtp, plp = (pools["spl"], pools["xp"], pools["mt"],
                               pools["plp"])
        pg = 0
        for w2 in range(NPW):
            spool = splp.tile([P, TPW * D], F32, tag="spool")
            for t0 in range(0, TPW, MAXCH):
                nt = min(MAXCH, TPW - t0)
                pg += 1
                gp = nc.gpsimd.dma_gather(
                    spool[:, t0 * D:(t0 + nt) * D]
                        .rearrange("p (c e) -> p c e", e=D),
                    s_hbm[:, :],
                    poolidx_sb[:, (w2 * TPW + t0) * 8:(w2 * TPW + t0 + nt) * 8],
                    nt * P, nt * P, D, single_packet=False,
                    queue_num=pg % NQ)
                add_dep_helper(gp.ins, s_write.ins, sync=True,
                               reason="pool gather after S write")
                add_dep_helper(gp.ins, s_write2.ins, sync=True,
                               reason="pool gather after S pad write")
            spq = splp.tile([P, TPW * K], BF16, tag="spq")
            nc.scalar.copy(
                out=spq[:].rearrange("p (c e) -> p c e", e=K),
                in_=spool[:].rearrange("p (c e) -> p c e", e=D)[:, :, :K])
            xp = xpp.tile([P, TPW * D], BF16)
            nc.sync.dma_start(
                out=xp[:].rearrange("p (t d) -> p t d", d=D),
                in_=xpool_d[w2 * TPW * P:(w2 + 1) * TPW * P, :]
                    .rearrange("(t p) d -> p t d", p=P))
            pps = plp.tile([P, D], F32)
            for t in range(TPW):
                tt = w2 * TPW + t
                mk = mtp.tile([P, GW], BF16, tag="mk")
                nc.vector.tensor_tensor(
                    out=mk[:], in0=bloc_sb[:, tt:tt + 1].to_broadcast([P, GW]),
                    in1=iota_sb[:, :GW], op=AluOp.is_equal)
                sst = mtp.tile([P, GW * K], BF16, tag="sst")
                nc.vector.tensor_tensor(
                    out=sst[:].rearrange("p (g k) -> p g k", k=K),
                    in0=spq[:, t * K:(t + 1) * K]
                        .unsqueeze(1).to_broadcast([P, GW, K]),
                    in1=mk[:].unsqueeze(2).to_broadcast([P, GW, K]),
                    op=AluOp.mult)
                nc.tensor.matmul(pps[:], lhsT=sst[:],
                                 rhs=xp[:, t * D:(t + 1) * D],
                                 start=(t == 0), stop=(t == TPW - 1))
            nc.vector.tensor_copy(out=pool_sb[:, w2 * D:(w2 + 1) * D],
                                  in_=pps[:])
        nc.sync.dma_start(out=pool_d[:], in_=pool_sb[:])
    nc.compile()
    if split:
        _split_waits(nc)
    return nc


# =========================================================================
# runners + glue
# =========================================================================

_TRACE = bool(int(os.environ.get("KERNEL_TRACE", "0")))
_LAST_EXEC_NS = {}
_LAST_WALL = {}


def _run_spmd(nc, in_maps, tag):
    import time
    core_ids = list(range(len(in_maps)))
    t0 = time.time()
    res = run_bass_kernel_spmd(nc, in_maps, core_ids, trace=_TRACE)
    _LAST_WALL[tag] = time.time() - t0
    if res.exec_time_ns is not None:
        _LAST_EXEC_NS[tag] = res.exec_time_ns
    return res.results


def make_in_maps_A(pr):
    maps = []
    for c in range(pr.n_cores):
        cb = pr.cbA.pack(dict(
            dinvT=pr.dinvT[c], srcg=pr.srcg16[c],
            bt=np.tile(pr.b1[None, :], (P, 1)).astype(np.float32),
            w1=np.tile(pr.W1.astype(NP_BF16), (1, 1)), iota=_iota_full(),
            dstloc=pr.dstlocT[c]))
        maps.append(dict(xT=pr.xT_A[c], cb=cb))
    return maps


def make_in_maps_B(pr, x_q, xs_q):
    D, K = pr.D, pr.K
    w2pad = np.zeros((P, D), NP_BF16)
    w2pad[:D, :K] = pr.W2.astype(NP_BF16)
    x_f = x_q.astype(np.float32)
    maps = []
    for c in range(pr.n_cores):
        cb = pr.cbB.pack(dict(
            dinvT=pr.dinvT[c], srcg=pr.srcg16[c],
            bt=np.tile(np.exp(pr.b2)[None, :], (P, 1)).astype(np.float32),
            poolidx=pr.poolidx16[c], w2=w2pad, iota=_iota_full(),
            dstloc=pr.dstlocT[c], bloc=pr.batchlocT[c]))
        pn = pr.poolnode[c]
        xp = np.zeros((pr.PT * P, D), np.float32)
        v = pn >= 0
        xp[v] = x_f[pn[v]]
        maps.append(dict(
            xT2=np.ascontiguousarray(xs_q[pr.perm[c]].T), cb=cb,
            xpool=xp.astype(NP_BF16)))
    return maps


def reduce_pool(pr, pool_outs):
    D, K = pr.D, pr.K
    pooled = np.zeros((pr.B, K, D), np.float64)
    for c in range(pr.n_cores):
        po = np.asarray(pool_outs[c]).astype(np.float64)
        gb = pr.gbase[c]
        blk = po.reshape(pr.GW, K, pr.NPW, D)
        for g_loc in range(pr.GW):
            for w2 in range(pr.NPW):
                g = gb + w2 * pr.GW + g_loc
                if g < pr.B:
                    pooled[g] += blk[g_loc, :, w2, :]
    return pooled.astype(np.float32)


def kernel(x_in, edge_index, batch, W1, b1, W2, b2):
    n_cores = 8
    pr = preprocess(x_in, edge_index, batch, W1, b1, W2, b2, n_cores)

    ncA = build_A(pr)
    resA = _run_spmd(ncA, make_in_maps_A(pr), "A")
    x_q = np.vstack([resA[c]["xout"] for c in range(n_cores)])
    xs_q = np.vstack([resA[c]["xsout"] for c in range(n_cores)])

    ncB = build_B(pr)
    resB = _run_spmd(ncB, make_in_maps_B(pr, x_q, xs_q), "B")
    return reduce_pool(pr, [resB[c]["pool"] for c in range(n_cores)])
